# revision 1
# baseline (speedup 1.0000x reference)
"""Trainium2 Bass kernel for nn_Net_76330158785143 (dense_cnn).

Pipeline per sample: per-sample 11x11 autocorrelation of channel 2 ->
conv5x5(1->32) relu -> maxpool2 -> conv5x5(32->64) relu -> maxpool2 ->
conv3x3(64->10) relu -> GAP -> log_softmax.

Sharding: pure data parallel, batch 8192 -> 1024 per core across 8 cores.
"""

import sys

sys.path.insert(0, "/opt/trn_rl_repo")

import numpy as np

import concourse.bacc as bacc
import concourse.mybir as mybir
from concourse.ap import AP
from concourse.tile import TileContext
from concourse.bass_utils import run_bass_kernel_spmd

F32 = mybir.dt.float32
BF16 = mybir.dt.bfloat16
ALU = mybir.AluOpType
ACTF = mybir.ActivationFunctionType
AXIS = mybir.AxisListType
POOLF = mybir.PoolFunctionType

DEBUG_TAPS = False
N_CORES = 8
B_FULL = 8192
B_CORE = B_FULL // N_CORES


def _build(nc, b_core):
    """Emit the full per-core program for b_core samples (multiple of 128)."""
    n_bt = b_core // 128  # 128-sample tiles

    x_d = nc.dram_tensor("x", [b_core, 3, 28, 28], F32, kind="ExternalInput")
    identp_d = nc.dram_tensor("identp", [128, 128], BF16, kind="ExternalInput")
    ident10p_d = nc.dram_tensor("ident10p", [16, 16], F32, kind="ExternalInput")
    w1p_d = nc.dram_tensor("w1p", [32, 32], BF16, kind="ExternalInput")
    w2p_d = nc.dram_tensor("w2p", [128, 448], BF16, kind="ExternalInput")
    w3p_d = nc.dram_tensor("w3p", [128, 56], BF16, kind="ExternalInput")
    b1p_d = nc.dram_tensor("b1p", [128, 1], F32, kind="ExternalInput")
    b2p_d = nc.dram_tensor("b2p", [64, 1], F32, kind="ExternalInput")
    b3p_d = nc.dram_tensor("b3p", [16, 1], F32, kind="ExternalInput")
    out_d = nc.dram_tensor("out", [b_core, 10], F32, kind="ExternalOutput")
    dbg = {}
    if DEBUG_TAPS:
        for nm, shp, dt in [
            ("dbg_corr", [128, 924], BF16), ("dbg_s25", [32, 21504], BF16),
            ("dbg_a1", [128, 288], BF16), ("dbg_py", [128, 72], BF16),
            ("dbg_dup4", [128, 4608], BF16), ("dbg_t3", [64, 512], F32),
            ("dbg_o2", [64, 128], BF16), ("dbg_lgr", [16, 128], F32),
            ("dbg_logitsb", [16, 128], F32),
        ]:
            dbg[nm] = nc.dram_tensor(nm, shp, dt, kind="ExternalOutput")

    with TileContext(nc) as tc:
        cpool_cm = tc.tile_pool(name="const", bufs=1)
        cpool = cpool_cm.__enter__()

        def _load_const(name, dram, shape, dtype):
            t = cpool.tile(shape, dtype, name=name + "_sb")
            f = int(np.prod(shape[1:]))
            nc.sync.dma_start(
                out=AP(t.tensor, 0, [[f, shape[0]], [1, f]]),
                in_=AP(dram, 0, [[f, shape[0]], [1, f]]),
            )
            return t

        # host-prearranged constants (see _prep_inputs)
        ident = _load_const("ident", identp_d, [128, 128], BF16)
        ident10 = _load_const("ident10", ident10p_d, [16, 16], F32)
        w1_sb = _load_const("w1p", w1p_d, [32, 32], BF16)
        w2_sb = _load_const("w2p", w2p_d, [128, 448], BF16)
        w3_sb = _load_const("w3p", w3p_d, [128, 56], BF16)
        b1_sb = _load_const("b1p", b1p_d, [128, 1], F32)
        b2_sb = _load_const("b2p", b2p_d, [64, 1], F32)
        b3_sb = _load_const("b3p", b3p_d, [16, 1], F32)

        with (
            tc.tile_pool(name="img", bufs=2) as imgpool,
            tc.tile_pool(name="tmpl", bufs=2) as tmplpool,
            tc.tile_pool(name="diag", bufs=4) as diagpool,
            tc.tile_pool(name="corr", bufs=2) as corrpool,
            tc.tile_pool(name="s25", bufs=2) as s25pool,
            tc.tile_pool(name="a1", bufs=3) as a1pool,
            tc.tile_pool(name="pool1", bufs=3) as p1pool,
            tc.tile_pool(name="dup4", bufs=2) as dup4pool,
            tc.tile_pool(name="t2", bufs=6) as t2pool,
            tc.tile_pool(name="o2", bufs=3) as o2pool,
            tc.tile_pool(name="l3", bufs=2) as l3pool,
            tc.tile_pool(name="sm", bufs=4) as smpool,
            tc.tile_pool(name="lgb", bufs=2) as lgbpool,
            tc.tile_pool(name="dscr", bufs=2, space="DRAM") as dscrpool,
            tc.tile_pool(name="pcorr", bufs=1, space="PSUM") as pcorr,
            tc.tile_pool(name="pc1", bufs=2, space="PSUM") as pc1,
            tc.tile_pool(name="pc2", bufs=2, space="PSUM") as pc2,
        ):
            pools = dict(
                imgpool=imgpool, tmplpool=tmplpool, diagpool=diagpool,
                corrpool=corrpool, s25pool=s25pool, a1pool=a1pool,
                p1pool=p1pool, dup4pool=dup4pool,
                t2pool=t2pool, o2pool=o2pool, l3pool=l3pool, smpool=smpool,
                lgbpool=lgbpool, dscrpool=dscrpool, pcorr=pcorr, pc1=pc1, pc2=pc2,
            )
            consts = dict(
                ident=ident, ident10=ident10, w1_sb=w1_sb, w2_sb=w2_sb,
                w3_sb=w3_sb, b1_sb=b1_sb, b2_sb=b2_sb, b3_sb=b3_sb,
            )
            for bt in range(n_bt):
                _do_btile(nc, bt, b_core, x_d, out_d, dbg, pools, consts)

        cpool_cm.__exit__(None, None, None)
    return nc


_CACHE = {}


def _get_nc(b_core):
    if b_core not in _CACHE:
        nc = bacc.Bacc("TRN2", target_bir_lowering=False, debug=False, num_devices=N_CORES)
        _build(nc, b_core)
        nc.compile()
        _CACHE[b_core] = nc
    return _CACHE[b_core]


def _prep_inputs(inputs):
    import ml_dtypes

    bf16 = ml_dtypes.bfloat16
    w1 = np.asarray(inputs["w1"], dtype=np.float32).reshape(32, 25)
    w2 = np.asarray(inputs["w2"], dtype=np.float32).reshape(64, 32, 25)
    w3 = np.asarray(inputs["w3"], dtype=np.float32).reshape(10, 64, 9)
    b1 = np.asarray(inputs["b1"], dtype=np.float32)
    b2 = np.asarray(inputs["b2"], dtype=np.float32)
    b3 = np.asarray(inputs["b3"], dtype=np.float32)

    w1p = np.ascontiguousarray(w1.T).astype(bf16)  # [25t, 32co] -> pad [32, 32]
    w1p = np.pad(w1p, ((0, 7), (0, 0)))
    w2p = np.zeros((128, 448), dtype=bf16)
    for t in range(25):
        r, slot = t % 4, t // 4
        # [ci, co] block at partitions 32r.., free slot*64..
        w2p[32 * r : 32 * r + 32, slot * 64 : slot * 64 + 64] = w2[:, :, t].T.astype(
            bf16
        )
    w3p = np.zeros((128, 56), dtype=bf16)
    for t in range(9):
        r, slot = t % 2, t // 2
        w3p[64 * r : 64 * r + 64, slot * 10 : slot * 10 + 10] = w3[:, :, t].T.astype(
            bf16
        )
    b1p = np.tile(b1, 4).reshape(128, 1)
    b2p = b2.reshape(64, 1)
    b3p = np.pad(b3, (0, 6)).reshape(16, 1)
    identp = np.eye(128, dtype=bf16)
    ident10p = np.eye(16, dtype=np.float32)
    return dict(
        identp=identp,
        ident10p=ident10p,
        w1p=w1p,
        w2p=w2p,
        w3p=w3p,
        b1p=b1p,
        b2p=b2p,
        b3p=b3p,
    )


def _run(inputs, b_core=B_CORE, trace=False):
    x = np.ascontiguousarray(np.asarray(inputs["x"], dtype=np.float32))
    consts = _prep_inputs(inputs)
    nc = _get_nc(b_core)
    in_maps = [
        {"x": x[i * b_core : (i + 1) * b_core], **consts} for i in range(N_CORES)
    ]
    res = run_bass_kernel_spmd(nc, in_maps, core_ids=list(range(N_CORES)), trace=trace)
    out = np.concatenate([res.results[i]["out"] for i in range(N_CORES)], axis=0)
    return out.astype(np.float32), res


def kernel(**inputs) -> np.ndarray:
    out, _ = _run(inputs)
    return out


def _do_btile(nc, bt, b_core, x_d, out_d, dbg, P, C):
    dscrpool = P["dscrpool"]
    imgpool = P["imgpool"]; tmplpool = P["tmplpool"]; diagpool = P["diagpool"]
    corrpool = P["corrpool"]; s25pool = P["s25pool"]; a1pool = P["a1pool"]
    p1pool = P["p1pool"]; dup4pool = P["dup4pool"]
    t2pool = P["t2pool"]; o2pool = P["o2pool"]; l3pool = P["l3pool"]
    smpool = P["smpool"]; lgbpool = P["lgbpool"]; pcorr = P["pcorr"]
    pc1 = P["pc1"]; pc2 = P["pc2"]
    ident = C["ident"]; ident10 = C["ident10"]; w1_sb = C["w1_sb"]
    w2_sb = C["w2_sb"]; w3_sb = C["w3_sb"]; b1_sb = C["b1_sb"]
    b2_sb = C["b2_sb"]; b3_sb = C["b3_sb"]

    # ---- load channel 2 into zero-padded 38x38, cast bf16 ----
    img = imgpool.tile([128, 38 * 38], BF16)
    nc.gpsimd.memset(img[:, :], 0.0)
    nc.gpsimd.dma_start(
        out=AP(img.tensor, 5 * 38 + 5, [[1444, 128], [38, 28], [1, 28]]),
        in_=AP(
            x_d,
            bt * 128 * 2352 + 2 * 784,
            [[2352, 128], [1, 784]],
        ),
    )
    # template = center 11x11 crop (rows/cols 8..18 of 28x28 content)
    tmpl = tmplpool.tile([128, 128], F32)
    nc.vector.tensor_copy(
        out=AP(tmpl.tensor, 0, [[128, 128], [1, 121]]),
        in_=AP(img.tensor, 13 * 38 + 13, [[1444, 128], [38, 11], [1, 11]]),
    )

    # ---- correlation: 121 accumulating diag matmuls ----
    ps_a = pcorr.tile([128, 392], F32, tag="corr_a")
    ps_b = pcorr.tile([128, 392], F32, tag="corr_b")
    for t in range(121):
        u, v = t // 11, t % 11
        dg = diagpool.tile([128, 128], BF16)
        nc.vector.tensor_scalar_mul(dg[:, :], ident[:, :], tmpl[:, t : t + 1])
        nc.tensor.matmul(
            ps_a[:, :],
            dg[:, :],
            AP(img.tensor, u * 38 + v, [[1444, 128], [38, 14], [1, 28]]),
            start=(t == 0),
            stop=(t == 120),
        )
        nc.tensor.matmul(
            ps_b[:, :],
            dg[:, :],
            AP(
                img.tensor,
                (u + 14) * 38 + v,
                [[1444, 128], [38, 14], [1, 28]],
            ),
            start=(t == 0),
            stop=(t == 120),
        )
    # corr in bf16, flat 784 + zero tail to 924 (shift window slack)
    corr = corrpool.tile([128, 924], BF16)
    nc.vector.tensor_copy(out=corr[:, 0:392], in_=ps_a[:, :])
    nc.vector.tensor_copy(out=corr[:, 392:784], in_=ps_b[:, :])
    nc.gpsimd.memset(corr[:, 784:924], 0.0)
    corr_d = dscrpool.tile([128, 924], BF16, tag="corr_d")
    nc.sync.dma_start(
        out=AP(corr_d.tensor, 0, [[924, 128], [1, 924]]),
        in_=corr[:, :],
    )
    if DEBUG_TAPS and bt == 0:
        nc.sync.dma_start(out=AP(dbg["dbg_corr"], 0, [[924, 128], [1, 924]]), in_=corr[:, :])

    logitsb = lgbpool.tile([16, 128], F32)

    for sub in range(4):  # 32-sample subchunks
        # ---- shift-replicate corr into 25 tap partitions ----
        # s25[p=(dy,dx), s*672 + j] = corr[s, dy*28+dx + j]
        out1p_d = dscrpool.tile([32, 32 * 144], BF16, tag="out1p_d")
        s25 = s25pool.tile([32, 32 * 672], BF16)
        _dbg1 = DEBUG_TAPS and bt == 0 and sub == 0
        for dy in range(5):
            nc.sync.dma_start(
                out=AP(s25.tensor, dy * 5 * 21504, [[21504, 5], [1, 21504]]),
                in_=AP(
                    corr_d.tensor,
                    sub * 32 * 924 + dy * 28,
                    [[1, 5], [924, 32], [1, 672]],
                ),
            )
        if _dbg1:
            nc.sync.dma_start(out=AP(dbg["dbg_s25"], 0, [[21504, 25], [1, 21504]]), in_=AP(s25.tensor, 0, [[21504, 25], [1, 21504]]))
        # ---- conv1: rounds of (4 samples x half-image), 4 col groups
        for q in range(8):
            for h in range(2):
                ps1 = pc1.tile([128, 288], F32, tag="ps1")
                for c in range(4):
                    s_loc = q * 4 + c
                    rhs = AP(
                        s25.tensor,
                        s_loc * 672 + h * 336,
                        [[21504, 25], [28, 12], [1, 24]],
                    )
                    nc.tensor.matmul(
                        ps1[32 * c : 32 * c + 32, :],
                        w1_sb[0:25, :],
                        rhs,
                        start=True,
                        stop=True,
                        tile_position=(0, 32 * c),
                    )
                # bias+relu+cast on ACT: a1 = relu(ps1 + b1)
                a1 = a1pool.tile([128, 288], BF16)
                nc.scalar.activation(
                    a1[:, :], ps1[:, :], ACTF.Relu, bias=b1_sb[:, 0:1]
                )
                if _dbg1 and q == 0 and h == 0:
                    nc.sync.dma_start(out=AP(dbg["dbg_a1"], 0, [[288, 128], [1, 288]]), in_=a1[:, :])
                # maxpool 2x2 (x then y)
                px = p1pool.tile([128, 144], BF16, tag="px")
                nc.vector.tensor_max(
                    px[:, :],
                    AP(a1.tensor, 0, [[288, 128], [24, 12], [2, 12]]),
                    AP(a1.tensor, 1, [[288, 128], [24, 12], [2, 12]]),
                )
                py = p1pool.tile([128, 72], BF16, tag="py")
                nc.vector.tensor_max(
                    py[:, :],
                    AP(px.tensor, 0, [[144, 128], [24, 6], [1, 12]]),
                    AP(px.tensor, 12, [[144, 128], [24, 6], [1, 12]]),
                )
                if _dbg1 and q == 0 and h == 0:
                    nc.sync.dma_start(out=AP(dbg["dbg_py"], 0, [[72, 128], [1, 72]]), in_=py[:, :])
                # consolidate to out1p_d [32ch, (s, 12, 12)] in DRAM
                nc.sync.dma_start(
                    out=AP(
                        out1p_d.tensor,
                        (q * 4) * 144 + h * 72,
                        [[144, 4], [4608, 32], [1, 72]],
                    ),
                    in_=py[:, :],
                )
        # ---- duplicate out1p to 4 row-group bases ----
        dup4 = dup4pool.tile([128, 32 * 144], BF16)
        for r in range(4):
            nc.sync.dma_start(
                out=dup4[32 * r : 32 * r + 32, :],
                in_=AP(
                    out1p_d.tensor,
                    0,
                    [[4608, 32], [1, 4608]],
                ),
            )
        if _dbg1:
            nc.sync.dma_start(out=AP(dbg["dbg_dup4"], 0, [[4608, 128], [1, 4608]]), in_=dup4[:, :])
        # ---- conv2: 25 taps as K=32 row-group tiles ----
        o2s = []
        for cc in range(4):  # 8-sample chunks, N=512
            psA = pc2.tile([128, 512], F32, tag="ps2a")
            psB = pc2.tile([128, 512], F32, tag="ps2b")
            for t in range(25):
                r = t % 4
                slot = t // 4
                dy, dx = t // 5, t % 5
                ps = psA if r < 2 else psB
                colb = 64 * (r % 2)
                rhs = AP(
                    dup4.tensor,
                    32 * r * 4608 + cc * 8 * 144 + dy * 12 + dx,
                    [[4608, 32], [144, 8], [12, 8], [1, 8]],
                )
                nc.tensor.matmul(
                    ps[colb : colb + 64, :],
                    w2_sb[32 * r : 32 * r + 32, slot * 64 : slot * 64 + 64],
                    rhs,
                    start=(t < 4),
                    stop=(t >= 21),
                    tile_position=(32 * r, colb),
                )
            t1 = t2pool.tile([64, 512], F32, tag="t1")
            nc.vector.tensor_scalar_add(t1[:, :], psA[0:64, :], b2_sb[:, 0:1])
            t2 = t2pool.tile([64, 512], F32, tag="t2")
            nc.vector.tensor_add(t2[:, :], t1[:, :], psA[64:128, :])
            t1b = t2pool.tile([64, 512], F32, tag="t1")
            nc.vector.tensor_add(t1b[:, :], t2[:, :], psB[0:64, :])
            t3 = t2pool.tile([64, 512], F32, tag="t3")
            nc.vector.tensor_add(t3[:, :], t1b[:, :], psB[64:128, :])
            if _dbg1 and cc == 0:
                nc.sync.dma_start(out=AP(dbg["dbg_t3"], 0, [[512, 64], [1, 512]]), in_=t3[:, :])
            # maxpool 2x2 (f32, pre-relu: relu commutes w/ max)
            qx = p1pool.tile([64, 256], F32, tag="qx")
            nc.vector.tensor_max(
                qx[:, :],
                AP(t3.tensor, 0, [[512, 64], [64, 8], [8, 8], [2, 4]]),
                AP(t3.tensor, 1, [[512, 64], [64, 8], [8, 8], [2, 4]]),
            )
            qy = p1pool.tile([64, 128], F32, tag="qy")
            nc.vector.tensor_max(
                qy[:, :],
                AP(qx.tensor, 0, [[256, 64], [32, 8], [8, 4], [1, 4]]),
                AP(qx.tensor, 4, [[256, 64], [32, 8], [8, 4], [1, 4]]),
            )
            o2 = o2pool.tile([64, 128], BF16)
            nc.scalar.activation(o2[:, :], qy[:, :], ACTF.Relu)
            if _dbg1 and cc == 0:
                nc.sync.dma_start(out=AP(dbg["dbg_o2"], 0, [[128, 64], [1, 128]]), in_=o2[:, :])
            o2s.append(o2)
        # ---- build l3 [128=(2dup,64ci), (32s,16)] ----
        l3 = l3pool.tile([128, 512], BF16)
        for cc in range(4):
            for r in range(2):
                nc.sync.dma_start(
                    out=l3[64 * r : 64 * r + 64, cc * 128 : cc * 128 + 128],
                    in_=o2s[cc][:, :],
                )
        # ---- conv3: 9 taps, 2 row tiles (K=64), N=128 ----
        ps3 = pc1.tile([64, 128], F32, tag="ps1")
        for t in range(9):
            r = t % 2
            slot = t // 2
            dy, dx = t // 3, t % 3
            rhs = AP(
                l3.tensor,
                64 * r * 512 + dy * 4 + dx,
                [[512, 64], [16, 32], [4, 2], [1, 2]],
            )
            nc.tensor.matmul(
                ps3[32 * r : 32 * r + 10, :],
                w3_sb[64 * r : 64 * r + 64, slot * 10 : slot * 10 + 10],
                rhs,
                start=(t < 2),
                stop=(t >= 7),
                tile_position=(64 * r, 32 * r),
            )
        # fold partials + bias, relu, GAP
        lg0 = smpool.tile([16, 128], F32, tag="lg0")
        nc.vector.tensor_scalar_add(lg0[0:10, :], ps3[0:10, :], b3_sb[0:10, 0:1])
        lg = smpool.tile([16, 128], F32, tag="lg")
        nc.vector.tensor_add(lg[0:10, :], lg0[0:10, :], ps3[32:42, :])
        lgr = smpool.tile([16, 128], F32, tag="lgr")
        nc.vector.tensor_scalar(
            lgr[0:10, :], lg[0:10, :], 0.0, 0.25, ALU.max, ALU.mult
        )
        if _dbg1:
            nc.sync.dma_start(out=AP(dbg["dbg_lgr"], 0, [[128, 16], [1, 128]]), in_=lgr[:, :])
        nc.vector.tensor_reduce(
            out=logitsb[0:10, sub * 32 : sub * 32 + 32],
            in_=AP(lgr.tensor, 0, [[128, 10], [4, 32], [1, 4]]),
            axis=AXIS.X,
            op=ALU.add,
        )

    # ---- transpose [10, 128] -> [128, 10], log_softmax, store ----
    psT = pc1.tile([128, 16], F32, tag="ps1")
    nc.tensor.transpose(
        psT[:, 0:10], logitsb[0:10, :], ident10[0:10, 0:10]
    )
    if DEBUG_TAPS and bt == 0:
        nc.sync.dma_start(out=AP(dbg["dbg_logitsb"], 0, [[128, 16], [1, 128]]), in_=logitsb[:, :])
    mx = smpool.tile([128, 1], F32, tag="mx")
    nc.vector.reduce_max(mx[:, :], psT[:, 0:10], axis=AXIS.X)
    hs = smpool.tile([128, 16], F32, tag="hs")
    nc.vector.tensor_scalar(
        hs[:, 0:10], psT[:, 0:10], mx[:, 0:1], None, ALU.subtract
    )
    ex = smpool.tile([128, 16], F32, tag="ex")
    nc.scalar.activation(ex[:, 0:10], hs[:, 0:10], ACTF.Exp)
    sm = smpool.tile([128, 1], F32, tag="sm")
    nc.vector.reduce_sum(sm[:, :], ex[:, 0:10], axis=AXIS.X)
    lsm = smpool.tile([128, 1], F32, tag="lsm")
    nc.scalar.activation(lsm[:, :], sm[:, :], ACTF.Ln)
    outt = smpool.tile([128, 16], F32, tag="outt")
    nc.vector.tensor_scalar(
        outt[:, 0:10], hs[:, 0:10], lsm[:, 0:1], None, ALU.subtract
    )
    nc.sync.dma_start(
        out=AP(out_d, bt * 1280, [[10, 128], [1, 10]]),
        in_=outt[:, 0:10],
    )




# revision 8
# speedup vs baseline: 1.1628x; 1.1628x over previous
"""Trainium2 Bass kernel for nn_Net_76330158785143 (dense_cnn).

Pipeline per sample: per-sample 11x11 autocorrelation of channel 2 ->
conv5x5(1->32) relu -> maxpool2 -> conv5x5(32->64) relu -> maxpool2 ->
conv3x3(64->10) relu -> GAP -> log_softmax.

Sharding: pure data parallel, batch 8192 -> 1024 per core across 8 cores.

v2 layout notes (per 128-sample btile, 4 subs of 32 samples each):
- sample-in-sub index s = 8*g + c  (g: partition-group 0..3, c: chunk 0..7)
- conv1: 4 samples stacked on PE rows (K=4x25=100), out partition m=4*co+g
- conv2: dy baked into dup_A row-groups (K=4x32=128), dx via rhs base offset;
  dy=4 row handled by dup_B (dx baked) + dup_C (tap (4,4), K=32)
- conv3: all 9 taps accumulate into one PSUM region (K=64)
"""

import sys

sys.path.insert(0, "/opt/trn_rl_repo")

import numpy as np

import concourse.bacc as bacc
import concourse.mybir as mybir
from concourse.ap import AP
from concourse.tile import TileContext
from concourse.bass_utils import run_bass_kernel_spmd

F32 = mybir.dt.float32
BF16 = mybir.dt.bfloat16
ALU = mybir.AluOpType
ACTF = mybir.ActivationFunctionType
AXIS = mybir.AxisListType

N_CORES = 8
B_FULL = 8192
B_CORE = B_FULL // N_CORES


def _build(nc, b_core):
    n_bt = b_core // 128

    x_d = nc.dram_tensor("x", [b_core, 3, 28, 28], F32, kind="ExternalInput")
    identp_d = nc.dram_tensor("identp", [128, 128], BF16, kind="ExternalInput")
    ident10p_d = nc.dram_tensor("ident10p", [16, 16], F32, kind="ExternalInput")
    w1x4_d = nc.dram_tensor("w1x4", [100, 128], BF16, kind="ExternalInput")
    b1x4_d = nc.dram_tensor("b1x4", [128, 1], F32, kind="ExternalInput")
    w2a_d = nc.dram_tensor("w2a", [128, 320], BF16, kind="ExternalInput")
    w2b4_d = nc.dram_tensor("w2b4", [128, 64], BF16, kind="ExternalInput")
    w2b1_d = nc.dram_tensor("w2b1", [32, 64], BF16, kind="ExternalInput")
    b2p_d = nc.dram_tensor("b2p", [64, 1], F32, kind="ExternalInput")
    w3n_d = nc.dram_tensor("w3n", [64, 96], BF16, kind="ExternalInput")
    b3q_d = nc.dram_tensor("b3q", [16, 1], F32, kind="ExternalInput")
    out_d = nc.dram_tensor("out", [b_core, 10], F32, kind="ExternalOutput")

    with TileContext(nc) as tc:
        cpool_cm = tc.tile_pool(name="const", bufs=1)
        cpool = cpool_cm.__enter__()

        def _load_const(name, dram, shape, dtype):
            t = cpool.tile(shape, dtype, name=name + "_sb")
            f = int(np.prod(shape[1:]))
            nc.sync.dma_start(
                out=AP(t.tensor, 0, [[f, shape[0]], [1, f]]),
                in_=AP(dram, 0, [[f, shape[0]], [1, f]]),
            )
            return t

        ident = _load_const("ident", identp_d, [128, 128], BF16)
        ident10 = _load_const("ident10", ident10p_d, [16, 16], F32)
        w1x4_sb = _load_const("w1x4", w1x4_d, [100, 128], BF16)
        b1x4_sb = _load_const("b1x4", b1x4_d, [128, 1], F32)
        w2a_sb = _load_const("w2a", w2a_d, [128, 320], BF16)
        w2b4_sb = _load_const("w2b4", w2b4_d, [128, 64], BF16)
        w2b1_sb = _load_const("w2b1", w2b1_d, [32, 64], BF16)
        b2p_sb = _load_const("b2p", b2p_d, [64, 1], F32)
        w3n_sb = _load_const("w3n", w3n_d, [64, 96], BF16)
        b3q_sb = _load_const("b3q", b3q_d, [16, 1], F32)
        # zero pad rows for corr_d tail (s25g shifted reads run past row 128)
        zpad = cpool.tile([8, 924], BF16, name="zpad_sb")
        nc.gpsimd.memset(zpad[:, :], 0.0)

        from contextlib import ExitStack

        with ExitStack() as stack:
            ent = stack.enter_context
            imgpool = ent(tc.tile_pool(name="img", bufs=2))
            tmplpool = ent(tc.tile_pool(name="tmpl", bufs=2))
            diagpool = ent(tc.tile_pool(name="diag", bufs=4))
            corrpool = ent(tc.tile_pool(name="corr", bufs=2))
            s25pool = ent(tc.tile_pool(name="s25", bufs=2))
            pxpool = ent(tc.tile_pool(name="px", bufs=2))
            pypool = ent(tc.tile_pool(name="py", bufs=2))
            pyactpool = ent(tc.tile_pool(name="pya", bufs=2))
            dupApool = ent(tc.tile_pool(name="dupA", bufs=2))
            dupBpool = ent(tc.tile_pool(name="dupB", bufs=2))
            dupCpool = ent(tc.tile_pool(name="dupC", bufs=2))
            qxpool = ent(tc.tile_pool(name="qx", bufs=2))
            qypool = ent(tc.tile_pool(name="qy", bufs=2))
            l3pool = ent(tc.tile_pool(name="l3", bufs=2))
            smpool = ent(tc.tile_pool(name="sm", bufs=4))
            lgbpool = ent(tc.tile_pool(name="lgb", bufs=2))
            dscrpool = ent(tc.tile_pool(name="dscr", bufs=2, space="DRAM"))
            pcorr = ent(tc.tile_pool(name="pcorr", bufs=1, space="PSUM"))
            pc1 = ent(tc.tile_pool(name="pc1", bufs=2, space="PSUM"))
            pc2 = ent(tc.tile_pool(name="pc2", bufs=2, space="PSUM"))
            pools = dict(
                imgpool=imgpool, tmplpool=tmplpool, diagpool=diagpool,
                corrpool=corrpool, s25pool=s25pool, pxpool=pxpool,
                pypool=pypool, pyactpool=pyactpool, dupApool=dupApool,
                dupBpool=dupBpool, dupCpool=dupCpool, qxpool=qxpool,
                qypool=qypool, l3pool=l3pool, smpool=smpool,
                lgbpool=lgbpool, dscrpool=dscrpool, pcorr=pcorr,
                pc1=pc1, pc2=pc2,
            )
            consts = dict(
                ident=ident, ident10=ident10, w1x4_sb=w1x4_sb,
                b1x4_sb=b1x4_sb, w2a_sb=w2a_sb, w2b4_sb=w2b4_sb,
                w2b1_sb=w2b1_sb, b2p_sb=b2p_sb, w3n_sb=w3n_sb,
                b3q_sb=b3q_sb, zpad=zpad,
            )
            # software pipeline: corr(b+1) issued before convs(b) so the
            # PE stream for convs overlaps the corr->s25g DMA latency
            state = {}
            for b in range(n_bt + 1):
                if b < n_bt:
                    state[b] = _corr_stage(nc, b, x_d, pools, consts)
                if b >= 1:
                    _conv_stage(nc, b - 1, state.pop(b - 1), out_d, pools, consts)

        cpool_cm.__exit__(None, None, None)
    return nc


def _corr_stage(nc, b, x_d, P, C):
    imgpool = P["imgpool"]; tmplpool = P["tmplpool"]; diagpool = P["diagpool"]
    corrpool = P["corrpool"]; dscrpool = P["dscrpool"]; pcorr = P["pcorr"]
    ident = C["ident"]; zpad = C["zpad"]

    # channel 2 into zero-padded 38x38, cast bf16
    img = imgpool.tile([128, 38 * 38], BF16)
    nc.gpsimd.memset(img[:, :], 0.0)
    nc.gpsimd.dma_start(
        out=AP(img.tensor, 5 * 38 + 5, [[1444, 128], [38, 28], [1, 28]]),
        in_=AP(x_d, b * 128 * 2352 + 2 * 784, [[2352, 128], [1, 784]]),
    )
    # template = center 11x11 crop
    tmpl = tmplpool.tile([128, 128], F32)
    nc.vector.tensor_copy(
        out=AP(tmpl.tensor, 0, [[128, 128], [1, 121]]),
        in_=AP(img.tensor, 13 * 38 + 13, [[1444, 128], [38, 11], [1, 11]]),
    )
    # 121 accumulating diag matmuls
    ps_a = pcorr.tile([128, 392], F32, tag="corr_a")
    ps_b = pcorr.tile([128, 392], F32, tag="corr_b")
    for t in range(121):
        u, v = t // 11, t % 11
        dg = diagpool.tile([128, 128], BF16)
        nc.vector.tensor_scalar_mul(dg[:, :], ident[:, :], tmpl[:, t : t + 1])
        nc.tensor.matmul(
            ps_a[:, :], dg[:, :],
            AP(img.tensor, u * 38 + v, [[1444, 128], [38, 14], [1, 28]]),
            start=(t == 0), stop=(t == 120),
        )
        nc.tensor.matmul(
            ps_b[:, :], dg[:, :],
            AP(img.tensor, (u + 14) * 38 + v, [[1444, 128], [38, 14], [1, 28]]),
            start=(t == 0), stop=(t == 120),
        )
    corr = corrpool.tile([128, 924], BF16)
    nc.vector.tensor_copy(out=corr[:, 0:392], in_=ps_a[:, :])
    nc.vector.tensor_copy(out=corr[:, 392:784], in_=ps_b[:, :])
    nc.gpsimd.memset(corr[:, 784:924], 0.0)
    corr_d = dscrpool.tile([136, 924], BF16, tag="corr_d")
    nc.sync.dma_start(
        out=AP(corr_d.tensor, 0, [[924, 128], [1, 924]]),
        in_=corr[:, :],
    )
    # zero tail rows (s25g shifted reads overrun into them)
    nc.sync.dma_start(
        out=AP(corr_d.tensor, 128 * 924, [[924, 8], [1, 924]]),
        in_=zpad[:, :],
    )
    return corr_d


def _conv_stage(nc, b, corr_d, out_d, P, C):
    s25pool = P["s25pool"]; pxpool = P["pxpool"]; pypool = P["pypool"]
    pyactpool = P["pyactpool"]; dupApool = P["dupApool"]
    dupBpool = P["dupBpool"]; dupCpool = P["dupCpool"]
    qxpool = P["qxpool"]; qypool = P["qypool"]; l3pool = P["l3pool"]
    smpool = P["smpool"]; lgbpool = P["lgbpool"]
    pc1 = P["pc1"]; pc2 = P["pc2"]
    w1x4_sb = C["w1x4_sb"]; b1x4_sb = C["b1x4_sb"]; w2a_sb = C["w2a_sb"]
    w2b4_sb = C["w2b4_sb"]; w2b1_sb = C["w2b1_sb"]; b2p_sb = C["b2p_sb"]
    w3n_sb = C["w3n_sb"]; b3q_sb = C["b3q_sb"]; ident10 = C["ident10"]

    logitsb = lgbpool.tile([16, 128], F32)

    for sub in range(4):
        # ---- im2col for conv1: s25g[20*dy+4*dx+g, c*924+j] =
        #      corr[32*sub+8*g+c, dy*28+dx + j]  (j runs the full 924) ----
        s25g = s25pool.tile([100, 7392], BF16)
        for dy in range(5):
            nc.sync.dma_start(
                out=s25g[20 * dy : 20 * dy + 20, :],
                in_=AP(
                    corr_d.tensor,
                    sub * 32 * 924 + dy * 28,
                    [[1, 5], [7392, 4], [1, 7392]],
                ),
            )
        # ---- conv1: 4 samples stacked (K=100), out m = 4*co+g ----
        px_all = pxpool.tile([128, 2304], BF16)
        for c in range(8):
            for h in range(2):
                ps1 = pc1.tile([128, 288], F32, tag="ps1")
                nc.tensor.matmul(
                    ps1[:, :],
                    w1x4_sb[0:100, :],
                    AP(
                        s25g.tensor,
                        c * 924 + h * 336,
                        [[7392, 100], [28, 12], [1, 24]],
                    ),
                    start=True, stop=True,
                )
                # maxpool x-pairs straight off PSUM (bias/relu commute)
                nc.vector.tensor_reduce(
                    out=px_all[:, c * 288 + h * 144 : c * 288 + h * 144 + 144],
                    in_=AP(ps1.tensor, 0, [[288, 128], [24, 12], [2, 12], [1, 2]]),
                    axis=AXIS.X,
                    op=ALU.max,
                )
        # maxpool y-pairs, then bias+relu once
        py_raw = pypool.tile([128, 1152], BF16)
        nc.vector.tensor_max(
            py_raw[:, :],
            AP(px_all.tensor, 0, [[2304, 128], [24, 96], [1, 12]]),
            AP(px_all.tensor, 12, [[2304, 128], [24, 96], [1, 12]]),
        )
        py_act = pyactpool.tile([128, 1204], BF16)
        nc.scalar.activation(
            py_act[:, 0:1152], py_raw[:, :], ACTF.Relu, bias=b1x4_sb[:, 0:1]
        )
        nc.gpsimd.memset(py_act[:, 1152:1204], 0.0)

        # ---- shifted dups for conv2 (SBUF->SBUF partition remap) ----
        # dup_A row-group r: shift dy=r   -> base 52-12r
        # dup_B row-group r: shift (4,r)  -> base 52-(48+r)
        # dup_C (32 rows):   shift (4,4)  -> in base +52
        dup_A = dupApool.tile([128, 4660], BF16)
        dup_B = dupBpool.tile([128, 4660], BF16)
        dup_C = dupCpool.tile([32, 4608], BF16)
        for r in range(4):
            nc.gpsimd.dma_start(
                out=AP(
                    dup_A.tensor,
                    32 * r * 4660 + (52 - 12 * r),
                    [[4660, 32], [1152, 4], [1, 1152]],
                ),
                in_=AP(py_act.tensor, 0, [[1204, 128], [1, 1152]]),
            )
            nc.gpsimd.dma_start(
                out=AP(
                    dup_B.tensor,
                    32 * r * 4660 + (52 - (48 + r)),
                    [[4660, 32], [1152, 4], [1, 1152]],
                ),
                in_=AP(py_act.tensor, 0, [[1204, 128], [1, 1152]]),
            )
        nc.gpsimd.dma_start(
            out=AP(dup_C.tensor, 0, [[4608, 32], [1152, 4], [1, 1152]]),
            in_=AP(py_act.tensor, 52, [[1204, 128], [1, 1152]]),
        )

        # ---- conv2: 7 matmuls per 8-sample chunk, all accumulate in PSUM ----
        qx_all = qxpool.tile([64, 1024], BF16)
        qy_all = qypool.tile([64, 512], BF16)
        for cc in range(4):
            ps2 = pc2.tile([64, 512], F32, tag="ps2")
            for dx in range(5):
                nc.tensor.matmul(
                    ps2[:, :],
                    w2a_sb[:, 64 * dx : 64 * dx + 64],
                    AP(
                        dup_A.tensor,
                        52 + cc * 1152 + dx,
                        [[4660, 128], [144, 8], [12, 8], [1, 8]],
                    ),
                    start=(dx == 0), stop=False,
                )
            nc.tensor.matmul(
                ps2[:, :],
                w2b4_sb[:, :],
                AP(
                    dup_B.tensor,
                    52 + cc * 1152,
                    [[4660, 128], [144, 8], [12, 8], [1, 8]],
                ),
                start=False, stop=False,
            )
            nc.tensor.matmul(
                ps2[:, :],
                w2b1_sb[:, :],
                AP(
                    dup_C.tensor,
                    cc * 1152,
                    [[4608, 32], [144, 8], [12, 8], [1, 8]],
                ),
                start=False, stop=True,
                tile_position=(0, 0),
            )
            # maxpool 2x2 off PSUM (pre-bias/relu; commutes)
            nc.vector.tensor_reduce(
                out=qx_all[:, cc * 256 : cc * 256 + 256],
                in_=AP(ps2.tensor, 0, [[512, 64], [8, 64], [2, 4], [1, 2]]),
                axis=AXIS.X,
                op=ALU.max,
            )
            nc.vector.tensor_max(
                qy_all[:, cc * 128 : cc * 128 + 128],
                AP(qx_all.tensor, cc * 256, [[1024, 64], [32, 8], [8, 4], [1, 4]]),
                AP(qx_all.tensor, cc * 256 + 4, [[1024, 64], [32, 8], [8, 4], [1, 4]]),
            )
        # bias+relu once -> conv3 input [64ci, (32s,16pix)]
        l3 = l3pool.tile([64, 512], BF16)
        nc.scalar.activation(
            l3[:, :], qy_all[:, :], ACTF.Relu, bias=b2p_sb[:, 0:1]
        )

        # ---- conv3: 9 taps, K=64, one PSUM region ----
        ps3 = pc1.tile([16, 128], F32, tag="ps1")
        for t in range(9):
            dy, dx = t // 3, t % 3
            nc.tensor.matmul(
                ps3[0:10, :],
                w3n_sb[0:64, 10 * t : 10 * t + 10],
                AP(l3.tensor, dy * 4 + dx, [[512, 64], [16, 32], [4, 2], [1, 2]]),
                start=(t == 0), stop=(t == 8),
            )
        # relu(0.25*x + 0.25*b3) then sum 4 pix = GAP of relu(x+b3)
        ga = smpool.tile([16, 128], F32, tag="ga")
        nc.scalar.activation(
            ga[0:10, :], ps3[0:10, :], ACTF.Relu, bias=b3q_sb[0:10, 0:1], scale=0.25
        )
        nc.vector.tensor_reduce(
            out=logitsb[0:10, sub * 32 : sub * 32 + 32],
            in_=AP(ga.tensor, 0, [[128, 10], [4, 32], [1, 4]]),
            axis=AXIS.X,
            op=ALU.add,
        )

    # ---- transpose [10,128] -> [128,10], log_softmax, store ----
    psT = pc1.tile([128, 16], F32, tag="ps1")
    nc.tensor.transpose(psT[:, 0:10], logitsb[0:10, :], ident10[0:10, 0:10])
    mx = smpool.tile([128, 1], F32, tag="mx")
    nc.vector.reduce_max(mx[:, :], psT[:, 0:10], axis=AXIS.X)
    hs = smpool.tile([128, 16], F32, tag="hs")
    nc.vector.tensor_scalar(hs[:, 0:10], psT[:, 0:10], mx[:, 0:1], None, ALU.subtract)
    ex = smpool.tile([128, 16], F32, tag="ex")
    nc.scalar.activation(ex[:, 0:10], hs[:, 0:10], ACTF.Exp)
    sm = smpool.tile([128, 1], F32, tag="sm")
    nc.vector.reduce_sum(sm[:, :], ex[:, 0:10], axis=AXIS.X)
    lsm = smpool.tile([128, 1], F32, tag="lsm")
    nc.scalar.activation(lsm[:, :], sm[:, :], ACTF.Ln)
    outt = smpool.tile([128, 16], F32, tag="outt")
    nc.vector.tensor_scalar(outt[:, 0:10], hs[:, 0:10], lsm[:, 0:1], None, ALU.subtract)
    nc.sync.dma_start(
        out=AP(out_d, b * 1280, [[10, 128], [1, 10]]),
        in_=outt[:, 0:10],
    )


_CACHE = {}


def _get_nc(b_core):
    if b_core not in _CACHE:
        nc = bacc.Bacc(
            "TRN2",
            target_bir_lowering=False,
            debug=False,
            num_devices=N_CORES,
            num_swdge_queues=2,
        )
        _build(nc, b_core)
        nc.compile()
        _CACHE[b_core] = nc
    return _CACHE[b_core]


def _prep_inputs(inputs):
    import ml_dtypes

    bf16 = ml_dtypes.bfloat16
    w1 = np.asarray(inputs["w1"], dtype=np.float32).reshape(32, 25)
    w2 = np.asarray(inputs["w2"], dtype=np.float32).reshape(64, 32, 5, 5)
    w3 = np.asarray(inputs["w3"], dtype=np.float32).reshape(10, 64, 9)
    b1 = np.asarray(inputs["b1"], dtype=np.float32)
    b2 = np.asarray(inputs["b2"], dtype=np.float32)
    b3 = np.asarray(inputs["b3"], dtype=np.float32)

    # conv1: w1x4[4*t+g, 4*co+g] = w1[co, t]
    w1x4 = np.zeros((100, 128), dtype=np.float32)
    for t in range(25):
        for g in range(4):
            w1x4[4 * t + g, 4 * np.arange(32) + g] = w1[:, t]
    b1x4 = np.zeros((128, 1), dtype=np.float32)
    for co in range(32):
        for g in range(4):
            b1x4[4 * co + g, 0] = b1[co]

    # conv2: w2a[32*dy+ci, 64*dx+co] = w2[co, ci, dy, dx] (dy 0..3)
    w2a = np.zeros((128, 320), dtype=np.float32)
    for dy in range(4):
        for dx in range(5):
            w2a[32 * dy : 32 * dy + 32, 64 * dx : 64 * dx + 64] = w2[:, :, dy, dx].T
    # w2b4[32*dx+ci, co] = w2[co, ci, 4, dx] (dx 0..3)
    w2b4 = np.zeros((128, 64), dtype=np.float32)
    for dx in range(4):
        w2b4[32 * dx : 32 * dx + 32, :] = w2[:, :, 4, dx].T
    w2b1 = np.ascontiguousarray(w2[:, :, 4, 4].T)

    # conv3: w3n[ci, 10*t+co] = w3[co, ci, t]
    w3n = np.zeros((64, 96), dtype=np.float32)
    for t in range(9):
        w3n[:, 10 * t : 10 * t + 10] = w3[:, :, t].T
    b3q = np.zeros((16, 1), dtype=np.float32)
    b3q[0:10, 0] = 0.25 * b3

    return dict(
        identp=np.eye(128, dtype=bf16),
        ident10p=np.eye(16, dtype=np.float32),
        w1x4=w1x4.astype(bf16),
        b1x4=b1x4,
        w2a=w2a.astype(bf16),
        w2b4=w2b4.astype(bf16),
        w2b1=w2b1.astype(bf16),
        b2p=b2.reshape(64, 1),
        w3n=w3n.astype(bf16),
        b3q=b3q,
    )


def _run(inputs, b_core=B_CORE, trace=False):
    x = np.ascontiguousarray(np.asarray(inputs["x"], dtype=np.float32))
    consts = _prep_inputs(inputs)
    nc = _get_nc(b_core)
    in_maps = [
        {"x": x[i * b_core : (i + 1) * b_core], **consts} for i in range(N_CORES)
    ]
    res = run_bass_kernel_spmd(nc, in_maps, core_ids=list(range(N_CORES)), trace=trace)
    out = np.concatenate([res.results[i]["out"] for i in range(N_CORES)], axis=0)
    return out.astype(np.float32), res


def kernel(**inputs) -> np.ndarray:
    out, _ = _run(inputs)
    return out


# revision 12
# speedup vs baseline: 1.2907x; 1.1101x over previous
"""Trainium2 Bass kernel for nn_Net_76330158785143 (dense_cnn).

Pipeline per sample: per-sample 11x11 autocorrelation of channel 2 ->
conv5x5(1->32) relu -> maxpool2 -> conv5x5(32->64) relu -> maxpool2 ->
conv3x3(64->10) relu -> GAP -> log_softmax.

Sharding: pure data parallel, batch 8192 -> 1024 per core across 8 cores.

v2 layout notes (per 128-sample btile, 4 subs of 32 samples each):
- sample-in-sub index s = 8*g + c  (g: partition-group 0..3, c: chunk 0..7)
- conv1: 4 samples stacked on PE rows (K=4x25=100), out partition m=4*co+g
- conv2: dy baked into dup_A row-groups (K=4x32=128), dx via rhs base offset;
  dy=4 row handled by dup_B (dx baked) + dup_C (tap (4,4), K=32)
- conv3: all 9 taps accumulate into one PSUM region (K=64)
"""

import sys

sys.path.insert(0, "/opt/trn_rl_repo")

import numpy as np

import concourse.bacc as bacc
import concourse.mybir as mybir
from concourse.ap import AP
from concourse.tile import TileContext
from concourse.bass_utils import run_bass_kernel_spmd

F32 = mybir.dt.float32
BF16 = mybir.dt.bfloat16
ALU = mybir.AluOpType
ACTF = mybir.ActivationFunctionType
AXIS = mybir.AxisListType

N_CORES = 8
B_FULL = 8192
B_CORE = B_FULL // N_CORES


def _build(nc, b_core):
    n_bt = b_core // 128

    x_d = nc.dram_tensor("x", [b_core, 3, 28, 28], F32, kind="ExternalInput")
    identp_d = nc.dram_tensor("identp", [128, 128], BF16, kind="ExternalInput")
    ident10p_d = nc.dram_tensor("ident10p", [16, 16], F32, kind="ExternalInput")
    w1x4_d = nc.dram_tensor("w1x4", [100, 128], BF16, kind="ExternalInput")
    b1x4_d = nc.dram_tensor("b1x4", [128, 1], F32, kind="ExternalInput")
    w2dx_d = nc.dram_tensor("w2dx", [128, 320], BF16, kind="ExternalInput")
    w2x4_d = nc.dram_tensor("w2x4", [32, 320], BF16, kind="ExternalInput")
    b2p_d = nc.dram_tensor("b2p", [64, 1], F32, kind="ExternalInput")
    w3n_d = nc.dram_tensor("w3n", [64, 96], BF16, kind="ExternalInput")
    b3q_d = nc.dram_tensor("b3q", [16, 1], F32, kind="ExternalInput")
    out_d = nc.dram_tensor("out", [b_core, 10], F32, kind="ExternalOutput")

    with TileContext(nc) as tc:
        cpool_cm = tc.tile_pool(name="const", bufs=1)
        cpool = cpool_cm.__enter__()

        def _load_const(name, dram, shape, dtype):
            t = cpool.tile(shape, dtype, name=name + "_sb")
            f = int(np.prod(shape[1:]))
            nc.sync.dma_start(
                out=AP(t.tensor, 0, [[f, shape[0]], [1, f]]),
                in_=AP(dram, 0, [[f, shape[0]], [1, f]]),
            )
            return t

        ident = _load_const("ident", identp_d, [128, 128], BF16)
        ident10 = _load_const("ident10", ident10p_d, [16, 16], F32)
        w1x4_sb = _load_const("w1x4", w1x4_d, [100, 128], BF16)
        b1x4_sb = _load_const("b1x4", b1x4_d, [128, 1], F32)
        w2dx_sb = _load_const("w2dx", w2dx_d, [128, 320], BF16)
        w2x4_sb = _load_const("w2x4", w2x4_d, [32, 320], BF16)
        b2p_sb = _load_const("b2p", b2p_d, [64, 1], F32)
        w3n_sb = _load_const("w3n", w3n_d, [64, 96], BF16)
        b3q_sb = _load_const("b3q", b3q_d, [16, 1], F32)
        # zero pad rows for corr_d tail (s25g shifted reads run past row 128)
        zpad = cpool.tile([8, 924], BF16, name="zpad_sb")
        nc.gpsimd.memset(zpad[:, :], 0.0)

        from contextlib import ExitStack

        with ExitStack() as stack:
            ent = stack.enter_context
            imgpool = ent(tc.tile_pool(name="img", bufs=2))
            tmplpool = ent(tc.tile_pool(name="tmpl", bufs=2))
            diagpool = ent(tc.tile_pool(name="diag", bufs=4))
            corrpool = ent(tc.tile_pool(name="corr", bufs=2))
            s25pool = ent(tc.tile_pool(name="s25", bufs=2))
            a1pool = ent(tc.tile_pool(name="a1", bufs=4))
            pxpool = ent(tc.tile_pool(name="px", bufs=2))
            pypool = ent(tc.tile_pool(name="py", bufs=2))
            dupApool = ent(tc.tile_pool(name="dupA", bufs=2))
            dup2pool = ent(tc.tile_pool(name="dup2", bufs=2))
            qxpool = ent(tc.tile_pool(name="qx", bufs=2))
            qypool = ent(tc.tile_pool(name="qy", bufs=2))
            l3pool = ent(tc.tile_pool(name="l3", bufs=2))
            smpool = ent(tc.tile_pool(name="sm", bufs=4))
            lgbpool = ent(tc.tile_pool(name="lgb", bufs=2))
            dscrpool = ent(tc.tile_pool(name="dscr", bufs=2, space="DRAM"))
            pcorr = ent(tc.tile_pool(name="pcorr", bufs=1, space="PSUM"))
            pc1 = ent(tc.tile_pool(name="pc1", bufs=2, space="PSUM"))
            pc2 = ent(tc.tile_pool(name="pc2", bufs=2, space="PSUM"))
            pc3 = ent(tc.tile_pool(name="pc3", bufs=1, space="PSUM"))
            pools = dict(
                imgpool=imgpool, tmplpool=tmplpool, diagpool=diagpool,
                corrpool=corrpool, s25pool=s25pool, a1pool=a1pool,
                pxpool=pxpool, pypool=pypool, dupApool=dupApool,
                dup2pool=dup2pool, qxpool=qxpool,
                qypool=qypool, l3pool=l3pool, smpool=smpool,
                lgbpool=lgbpool, dscrpool=dscrpool, pcorr=pcorr,
                pc1=pc1, pc2=pc2, pc3=pc3,
            )
            consts = dict(
                ident=ident, ident10=ident10, w1x4_sb=w1x4_sb,
                b1x4_sb=b1x4_sb, w2dx_sb=w2dx_sb, w2x4_sb=w2x4_sb,
                b2p_sb=b2p_sb, w3n_sb=w3n_sb,
                b3q_sb=b3q_sb, zpad=zpad,
            )
            # software pipeline: corr(b+1) issued before convs(b) so the
            # PE stream for convs overlaps the corr->s25g DMA latency
            state = {}
            for b in range(n_bt + 1):
                if b < n_bt:
                    state[b] = _corr_stage(nc, b, x_d, pools, consts)
                if b >= 1:
                    _conv_stage(nc, b - 1, state.pop(b - 1), out_d, pools, consts)

        cpool_cm.__exit__(None, None, None)
    return nc


def _corr_stage(nc, b, x_d, P, C):
    imgpool = P["imgpool"]; tmplpool = P["tmplpool"]; diagpool = P["diagpool"]
    corrpool = P["corrpool"]; dscrpool = P["dscrpool"]; pcorr = P["pcorr"]
    ident = C["ident"]; zpad = C["zpad"]

    # channel 2 into zero-padded 38x38, cast bf16
    img = imgpool.tile([128, 38 * 38], BF16)
    nc.gpsimd.memset(img[:, :], 0.0)
    nc.gpsimd.dma_start(
        out=AP(img.tensor, 5 * 38 + 5, [[1444, 128], [38, 28], [1, 28]]),
        in_=AP(x_d, b * 128 * 2352 + 2 * 784, [[2352, 128], [1, 784]]),
    )
    # template = center 11x11 crop
    tmpl = tmplpool.tile([128, 128], F32)
    nc.vector.tensor_copy(
        out=AP(tmpl.tensor, 0, [[128, 128], [1, 121]]),
        in_=AP(img.tensor, 13 * 38 + 13, [[1444, 128], [38, 11], [1, 11]]),
    )
    # 121 accumulating diag matmuls
    ps_a = pcorr.tile([128, 392], F32, tag="corr_a")
    ps_b = pcorr.tile([128, 392], F32, tag="corr_b")
    for t in range(121):
        u, v = t // 11, t % 11
        dg = diagpool.tile([128, 128], BF16)
        nc.vector.tensor_scalar_mul(dg[:, :], ident[:, :], tmpl[:, t : t + 1])
        nc.tensor.matmul(
            ps_a[:, :], dg[:, :],
            AP(img.tensor, u * 38 + v, [[1444, 128], [38, 14], [1, 28]]),
            start=(t == 0), stop=(t == 120),
        )
        nc.tensor.matmul(
            ps_b[:, :], dg[:, :],
            AP(img.tensor, (u + 14) * 38 + v, [[1444, 128], [38, 14], [1, 28]]),
            start=(t == 0), stop=(t == 120),
        )
    corr = corrpool.tile([128, 924], BF16)
    nc.vector.tensor_copy(out=corr[:, 0:392], in_=ps_a[:, :])
    nc.vector.tensor_copy(out=corr[:, 392:784], in_=ps_b[:, :])
    nc.gpsimd.memset(corr[:, 784:924], 0.0)
    corr_d = dscrpool.tile([136, 924], BF16, tag="corr_d")
    nc.sync.dma_start(
        out=AP(corr_d.tensor, 0, [[924, 128], [1, 924]]),
        in_=corr[:, :],
    )
    # zero tail rows (s25g shifted reads overrun into them)
    nc.sync.dma_start(
        out=AP(corr_d.tensor, 128 * 924, [[924, 8], [1, 924]]),
        in_=zpad[:, :],
    )
    return corr_d


def _conv_stage(nc, b, corr_d, out_d, P, C):
    logitsb = P["lgbpool"].tile([16, 128], F32)
    # sub-level software pipeline: stage A produces dup tiles for sub s
    # while stage B consumes sub s-1, so conv1(s+1) PE work hides the
    # dup DMA latency in front of conv2(s).
    dups = {}
    for s in range(5):
        if s < 4:
            dups[s] = _conv_a(nc, b, s, corr_d, P, C)
        if s >= 1:
            _conv_b(nc, b, s - 1, dups.pop(s - 1), logitsb, P, C)
    _softmax_out(nc, b, logitsb, out_d, P, C)


def _conv_a(nc, b, sub, corr_d, P, C):
    """s25g im2col -> conv1 -> relu+bias -> maxpool -> shifted dup tiles."""
    w1x4_sb = C["w1x4_sb"]; b1x4_sb = C["b1x4_sb"]
    # s25g[20*dy+4*dx+g, c*924+j] = corr[32*sub+8*g+c, dy*28+dx + j]
    s25g = P["s25pool"].tile([100, 7392], BF16)
    for dy in range(5):
        nc.sync.dma_start(
            out=s25g[20 * dy : 20 * dy + 20, :],
            in_=AP(
                corr_d.tensor,
                sub * 32 * 924 + dy * 28,
                [[1, 5], [7392, 4], [1, 7392]],
            ),
        )
    px_all = P["pxpool"].tile([128, 2304], BF16)
    for c in range(8):
        for h in range(2):
            ps1 = P["pc1"].tile([128, 288], F32, tag="ps1")
            nc.tensor.matmul(
                ps1[:, :],
                w1x4_sb[0:100, :],
                AP(
                    s25g.tensor,
                    c * 924 + h * 336,
                    [[7392, 100], [28, 12], [1, 24]],
                ),
                start=True, stop=True,
            )
            a1 = P["a1pool"].tile([128, 288], BF16)
            nc.scalar.activation(
                a1[:, :], ps1[:, :], ACTF.Relu, bias=b1x4_sb[:, 0:1]
            )
            # maxpool x-pairs
            nc.vector.tensor_reduce(
                out=px_all[:, c * 288 + h * 144 : c * 288 + h * 144 + 144],
                in_=AP(a1.tensor, 0, [[288, 128], [24, 12], [2, 12], [1, 2]]),
                axis=AXIS.X,
                op=ALU.max,
            )
    # maxpool y-pairs -> pooled [128=(4co+g), (c, 12, 12)]
    py_all = P["pypool"].tile([128, 1204], BF16)
    nc.vector.tensor_max(
        py_all[:, 0:1152],
        AP(px_all.tensor, 0, [[2304, 128], [24, 96], [1, 12]]),
        AP(px_all.tensor, 12, [[2304, 128], [24, 96], [1, 12]]),
    )
    nc.gpsimd.memset(py_all[:, 1152:1204], 0.0)
    # shifted dups (partition remap to [32ci, ...]):
    # dup_A row-group r bakes dx=r (shift r elems); dup2 bakes dx=4
    dup_A = P["dupApool"].tile([128, 4612], BF16)
    dup2 = P["dup2pool"].tile([32, 4608], BF16)
    for r in range(4):
        nc.gpsimd.dma_start(
            out=AP(
                dup_A.tensor,
                32 * r * 4612 + (4 - r),
                [[4612, 32], [1152, 4], [1, 1152]],
            ),
            in_=AP(py_all.tensor, 0, [[1204, 128], [1, 1152]]),
        )
    nc.gpsimd.dma_start(
        out=AP(dup2.tensor, 0, [[4608, 32], [1152, 4], [1, 1152]]),
        in_=AP(py_all.tensor, 4, [[1204, 128], [1, 1152]]),
    )
    return dup_A, dup2


def _conv_b(nc, b, sub, dup_pair, logitsb, P, C):
    """conv2 (dy via rhs offset, dx via dup row-groups) -> pool -> conv3 -> GAP."""
    dup_A, dup2 = dup_pair
    w2dx_sb = C["w2dx_sb"]; w2x4_sb = C["w2x4_sb"]; b2p_sb = C["b2p_sb"]
    w3n_sb = C["w3n_sb"]; b3q_sb = C["b3q_sb"]

    qx_all = P["qxpool"].tile([64, 1024], BF16)
    qy_all = P["qypool"].tile([64, 512], BF16)
    for cc in range(4):
        ps2 = P["pc2"].tile([64, 512], F32, tag="ps2")
        for dy in range(5):
            nc.tensor.matmul(
                ps2[:, :],
                w2dx_sb[:, 64 * dy : 64 * dy + 64],
                AP(
                    dup_A.tensor,
                    4 + cc * 1152 + dy * 12,
                    [[4612, 128], [144, 8], [12, 8], [1, 8]],
                ),
                start=(dy == 0), stop=False,
            )
        for dy in range(5):
            nc.tensor.matmul(
                ps2[:, :],
                w2x4_sb[:, 64 * dy : 64 * dy + 64],
                AP(
                    dup2.tensor,
                    cc * 1152 + dy * 12,
                    [[4608, 32], [144, 8], [12, 8], [1, 8]],
                ),
                start=False, stop=(dy == 4),
                tile_position=(0, 0),
            )
        # maxpool 2x2 off PSUM (pre-bias/relu; commutes)
        nc.vector.tensor_reduce(
            out=qx_all[:, cc * 256 : cc * 256 + 256],
            in_=AP(ps2.tensor, 0, [[512, 64], [8, 64], [2, 4], [1, 2]]),
            axis=AXIS.X,
            op=ALU.max,
        )
        nc.vector.tensor_max(
            qy_all[:, cc * 128 : cc * 128 + 128],
            AP(qx_all.tensor, cc * 256, [[1024, 64], [32, 8], [8, 4], [1, 4]]),
            AP(qx_all.tensor, cc * 256 + 4, [[1024, 64], [32, 8], [8, 4], [1, 4]]),
        )
    # bias+relu once -> conv3 input [64ci, (32s,16pix)]
    l3 = P["l3pool"].tile([64, 512], BF16)
    nc.scalar.activation(l3[:, :], qy_all[:, :], ACTF.Relu, bias=b2p_sb[:, 0:1])

    # conv3: 9 taps, K=64, one PSUM region
    ps3 = P["pc3"].tile([16, 128], F32, tag="ps3")
    for t in range(9):
        dy, dx = t // 3, t % 3
        nc.tensor.matmul(
            ps3[0:10, :],
            w3n_sb[0:64, 10 * t : 10 * t + 10],
            AP(l3.tensor, dy * 4 + dx, [[512, 64], [16, 32], [4, 2], [1, 2]]),
            start=(t == 0), stop=(t == 8),
        )
    # relu(0.25*x + 0.25*b3) then sum 4 pix = GAP of relu(x+b3)
    ga = P["smpool"].tile([16, 128], F32, tag="ga")
    nc.scalar.activation(
        ga[0:10, :], ps3[0:10, :], ACTF.Relu, bias=b3q_sb[0:10, 0:1], scale=0.25
    )
    nc.vector.tensor_reduce(
        out=logitsb[0:10, sub * 32 : sub * 32 + 32],
        in_=AP(ga.tensor, 0, [[128, 10], [4, 32], [1, 4]]),
        axis=AXIS.X,
        op=ALU.add,
    )


def _softmax_out(nc, b, logitsb, out_d, P, C):
    smpool = P["smpool"]
    psT = P["pc3"].tile([128, 16], F32, tag="psT")
    nc.tensor.transpose(psT[:, 0:10], logitsb[0:10, :], C["ident10"][0:10, 0:10])
    mx = smpool.tile([128, 1], F32, tag="mx")
    nc.vector.reduce_max(mx[:, :], psT[:, 0:10], axis=AXIS.X)
    hs = smpool.tile([128, 16], F32, tag="hs")
    nc.vector.tensor_scalar(hs[:, 0:10], psT[:, 0:10], mx[:, 0:1], None, ALU.subtract)
    ex = smpool.tile([128, 16], F32, tag="ex")
    nc.scalar.activation(ex[:, 0:10], hs[:, 0:10], ACTF.Exp)
    sm = smpool.tile([128, 1], F32, tag="sm")
    nc.vector.reduce_sum(sm[:, :], ex[:, 0:10], axis=AXIS.X)
    lsm = smpool.tile([128, 1], F32, tag="lsm")
    nc.scalar.activation(lsm[:, :], sm[:, :], ACTF.Ln)
    outt = smpool.tile([128, 16], F32, tag="outt")
    nc.vector.tensor_scalar(outt[:, 0:10], hs[:, 0:10], lsm[:, 0:1], None, ALU.subtract)
    nc.sync.dma_start(
        out=AP(out_d, b * 1280, [[10, 128], [1, 10]]),
        in_=outt[:, 0:10],
    )


_CACHE = {}


def _get_nc(b_core):
    if b_core not in _CACHE:
        nc = bacc.Bacc(
            "TRN2",
            target_bir_lowering=False,
            debug=False,
            num_devices=N_CORES,
            num_swdge_queues=2,
        )
        _build(nc, b_core)
        nc.compile()
        _CACHE[b_core] = nc
    return _CACHE[b_core]


def _prep_inputs(inputs):
    import ml_dtypes

    bf16 = ml_dtypes.bfloat16
    w1 = np.asarray(inputs["w1"], dtype=np.float32).reshape(32, 25)
    w2 = np.asarray(inputs["w2"], dtype=np.float32).reshape(64, 32, 5, 5)
    w3 = np.asarray(inputs["w3"], dtype=np.float32).reshape(10, 64, 9)
    b1 = np.asarray(inputs["b1"], dtype=np.float32)
    b2 = np.asarray(inputs["b2"], dtype=np.float32)
    b3 = np.asarray(inputs["b3"], dtype=np.float32)

    # conv1: w1x4[4*t+g, 4*co+g] = w1[co, t]
    w1x4 = np.zeros((100, 128), dtype=np.float32)
    for t in range(25):
        for g in range(4):
            w1x4[4 * t + g, 4 * np.arange(32) + g] = w1[:, t]
    b1x4 = np.zeros((128, 1), dtype=np.float32)
    for co in range(32):
        for g in range(4):
            b1x4[4 * co + g, 0] = b1[co]

    # conv2: w2dx[32*r+ci, 64*dy+co] = w2[co, ci, dy, r] (r=dx 0..3)
    w2dx = np.zeros((128, 320), dtype=np.float32)
    for r in range(4):
        for dy in range(5):
            w2dx[32 * r : 32 * r + 32, 64 * dy : 64 * dy + 64] = w2[:, :, dy, r].T
    # w2x4[ci, 64*dy+co] = w2[co, ci, dy, 4]
    w2x4 = np.zeros((32, 320), dtype=np.float32)
    for dy in range(5):
        w2x4[:, 64 * dy : 64 * dy + 64] = w2[:, :, dy, 4].T

    # conv3: w3n[ci, 10*t+co] = w3[co, ci, t]
    w3n = np.zeros((64, 96), dtype=np.float32)
    for t in range(9):
        w3n[:, 10 * t : 10 * t + 10] = w3[:, :, t].T
    b3q = np.zeros((16, 1), dtype=np.float32)
    b3q[0:10, 0] = 0.25 * b3

    return dict(
        identp=np.eye(128, dtype=bf16),
        ident10p=np.eye(16, dtype=np.float32),
        w1x4=w1x4.astype(bf16),
        b1x4=b1x4,
        w2dx=w2dx.astype(bf16),
        w2x4=w2x4.astype(bf16),
        b2p=b2.reshape(64, 1),
        w3n=w3n.astype(bf16),
        b3q=b3q,
    )


def _run(inputs, b_core=B_CORE, trace=False):
    x = np.ascontiguousarray(np.asarray(inputs["x"], dtype=np.float32))
    consts = _prep_inputs(inputs)
    nc = _get_nc(b_core)
    in_maps = [
        {"x": x[i * b_core : (i + 1) * b_core], **consts} for i in range(N_CORES)
    ]
    res = run_bass_kernel_spmd(nc, in_maps, core_ids=list(range(N_CORES)), trace=trace)
    out = np.concatenate([res.results[i]["out"] for i in range(N_CORES)], axis=0)
    return out.astype(np.float32), res


def kernel(**inputs) -> np.ndarray:
    out, _ = _run(inputs)
    return out


# revision 13
# speedup vs baseline: 1.8539x; 1.4363x over previous
"""Trainium2 Bass kernel for nn_Net_76330158785143 (dense_cnn).

Pipeline per sample: per-sample 11x11 autocorrelation of channel 2 ->
conv5x5(1->32) relu -> maxpool2 -> conv5x5(32->64) relu -> maxpool2 ->
conv3x3(64->10) relu -> GAP -> log_softmax.

Sharding: pure data parallel, batch 8192 -> 1024 per core across 8 cores.

v2 layout notes (per 128-sample btile, 4 subs of 32 samples each):
- sample-in-sub index s = 8*g + c  (g: partition-group 0..3, c: chunk 0..7)
- conv1: 4 samples stacked on PE rows (K=4x25=100), out partition m=4*co+g
- conv2: dy baked into dup_A row-groups (K=4x32=128), dx via rhs base offset;
  dy=4 row handled by dup_B (dx baked) + dup_C (tap (4,4), K=32)
- conv3: all 9 taps accumulate into one PSUM region (K=64)
"""

import sys

sys.path.insert(0, "/opt/trn_rl_repo")

import numpy as np

import concourse.bacc as bacc
import concourse.mybir as mybir
from concourse.ap import AP
from concourse.tile import TileContext
from concourse.bass_utils import run_bass_kernel_spmd

F32 = mybir.dt.float32
BF16 = mybir.dt.bfloat16
ALU = mybir.AluOpType
ACTF = mybir.ActivationFunctionType
AXIS = mybir.AxisListType

N_CORES = 8
B_FULL = 8192
B_CORE = B_FULL // N_CORES


def _build(nc, b_core):
    n_bt = b_core // 128

    x_d = nc.dram_tensor("x", [b_core, 3, 28, 28], F32, kind="ExternalInput")
    identp_d = nc.dram_tensor("identp", [128, 128], BF16, kind="ExternalInput")
    ident10p_d = nc.dram_tensor("ident10p", [16, 16], F32, kind="ExternalInput")
    w1x4_d = nc.dram_tensor("w1x4", [100, 128], BF16, kind="ExternalInput")
    b1x4_d = nc.dram_tensor("b1x4", [128, 1], F32, kind="ExternalInput")
    w2dx_d = nc.dram_tensor("w2dx", [128, 320], BF16, kind="ExternalInput")
    w2x4_d = nc.dram_tensor("w2x4", [32, 320], BF16, kind="ExternalInput")
    b2p_d = nc.dram_tensor("b2p", [64, 1], F32, kind="ExternalInput")
    w3n_d = nc.dram_tensor("w3n", [64, 96], BF16, kind="ExternalInput")
    b3q_d = nc.dram_tensor("b3q", [16, 1], F32, kind="ExternalInput")
    out_d = nc.dram_tensor("out", [b_core, 10], F32, kind="ExternalOutput")

    with TileContext(nc) as tc:
        cpool_cm = tc.tile_pool(name="const", bufs=1)
        cpool = cpool_cm.__enter__()

        def _load_const(name, dram, shape, dtype):
            t = cpool.tile(shape, dtype, name=name + "_sb")
            f = int(np.prod(shape[1:]))
            nc.sync.dma_start(
                out=AP(t.tensor, 0, [[f, shape[0]], [1, f]]),
                in_=AP(dram, 0, [[f, shape[0]], [1, f]]),
            )
            return t

        ident = _load_const("ident", identp_d, [128, 128], BF16)
        ident10 = _load_const("ident10", ident10p_d, [16, 16], F32)
        w1x4_sb = _load_const("w1x4", w1x4_d, [100, 128], BF16)
        b1x4_sb = _load_const("b1x4", b1x4_d, [128, 1], F32)
        w2dx_sb = _load_const("w2dx", w2dx_d, [128, 320], BF16)
        w2x4_sb = _load_const("w2x4", w2x4_d, [32, 320], BF16)
        b2p_sb = _load_const("b2p", b2p_d, [64, 1], F32)
        w3n_sb = _load_const("w3n", w3n_d, [64, 96], BF16)
        b3q_sb = _load_const("b3q", b3q_d, [16, 1], F32)
        # zero pad rows for corr_d tail (s25g shifted reads run past row 128)
        zpad = cpool.tile([8, 924], BF16, name="zpad_sb")
        nc.gpsimd.memset(zpad[:, :], 0.0)

        from contextlib import ExitStack

        with ExitStack() as stack:
            ent = stack.enter_context
            imgpool = ent(tc.tile_pool(name="img", bufs=2))
            tmplpool = ent(tc.tile_pool(name="tmpl", bufs=2))
            diagpool = ent(tc.tile_pool(name="diag", bufs=4))
            corrpool = ent(tc.tile_pool(name="corr", bufs=2))
            s25pool = ent(tc.tile_pool(name="s25", bufs=3))
            pxpool = ent(tc.tile_pool(name="px", bufs=3))
            pyrpool = ent(tc.tile_pool(name="pyr", bufs=2))
            pypool = ent(tc.tile_pool(name="py", bufs=3))
            dupApool = ent(tc.tile_pool(name="dupA", bufs=3))
            dup2pool = ent(tc.tile_pool(name="dup2", bufs=3))
            o2pool = ent(tc.tile_pool(name="o2", bufs=2))
            qxpool = ent(tc.tile_pool(name="qx", bufs=2))
            l3pool = ent(tc.tile_pool(name="l3", bufs=2))
            smpool = ent(tc.tile_pool(name="sm", bufs=4))
            lgbpool = ent(tc.tile_pool(name="lgb", bufs=2))
            dscrpool = ent(tc.tile_pool(name="dscr", bufs=2, space="DRAM"))
            pcorr = ent(tc.tile_pool(name="pcorr", bufs=1, space="PSUM"))
            pc1 = ent(tc.tile_pool(name="pc1", bufs=2, space="PSUM"))
            pc2 = ent(tc.tile_pool(name="pc2", bufs=2, space="PSUM"))
            pc3 = ent(tc.tile_pool(name="pc3", bufs=1, space="PSUM"))
            pools = dict(
                imgpool=imgpool, tmplpool=tmplpool, diagpool=diagpool,
                corrpool=corrpool, s25pool=s25pool,
                pxpool=pxpool, pyrpool=pyrpool, pypool=pypool,
                dupApool=dupApool, dup2pool=dup2pool, o2pool=o2pool,
                qxpool=qxpool, l3pool=l3pool, smpool=smpool,
                lgbpool=lgbpool, dscrpool=dscrpool, pcorr=pcorr,
                pc1=pc1, pc2=pc2, pc3=pc3,
            )
            consts = dict(
                ident=ident, ident10=ident10, w1x4_sb=w1x4_sb,
                b1x4_sb=b1x4_sb, w2dx_sb=w2dx_sb, w2x4_sb=w2x4_sb,
                b2p_sb=b2p_sb, w3n_sb=w3n_sb,
                b3q_sb=b3q_sb, zpad=zpad,
            )
            # software pipeline: corr(b+1) issued before convs(b) so the
            # PE stream for convs overlaps the corr->s25g DMA latency
            state = {}
            for b in range(n_bt + 1):
                if b < n_bt:
                    state[b] = _corr_stage(nc, b, x_d, pools, consts)
                if b >= 1:
                    _conv_stage(nc, b - 1, state.pop(b - 1), out_d, pools, consts)

        cpool_cm.__exit__(None, None, None)
    return nc


def _corr_stage(nc, b, x_d, P, C):
    imgpool = P["imgpool"]; tmplpool = P["tmplpool"]; diagpool = P["diagpool"]
    corrpool = P["corrpool"]; dscrpool = P["dscrpool"]; pcorr = P["pcorr"]
    ident = C["ident"]; zpad = C["zpad"]

    # channel 2 into zero-padded 38x38, cast bf16
    img = imgpool.tile([128, 38 * 38], BF16)
    nc.gpsimd.memset(img[:, :], 0.0)
    nc.gpsimd.dma_start(
        out=AP(img.tensor, 5 * 38 + 5, [[1444, 128], [38, 28], [1, 28]]),
        in_=AP(x_d, b * 128 * 2352 + 2 * 784, [[2352, 128], [1, 784]]),
    )
    # template = center 11x11 crop
    tmpl = tmplpool.tile([128, 128], F32)
    nc.vector.tensor_copy(
        out=AP(tmpl.tensor, 0, [[128, 128], [1, 121]]),
        in_=AP(img.tensor, 13 * 38 + 13, [[1444, 128], [38, 11], [1, 11]]),
    )
    # 121 accumulating diag matmuls
    ps_a = pcorr.tile([128, 392], F32, tag="corr_a")
    ps_b = pcorr.tile([128, 392], F32, tag="corr_b")
    for t in range(121):
        u, v = t // 11, t % 11
        dg = diagpool.tile([128, 128], BF16)
        nc.vector.tensor_scalar_mul(dg[:, :], ident[:, :], tmpl[:, t : t + 1])
        nc.tensor.matmul(
            ps_a[:, :], dg[:, :],
            AP(img.tensor, u * 38 + v, [[1444, 128], [38, 14], [1, 28]]),
            start=(t == 0), stop=(t == 120),
        )
        nc.tensor.matmul(
            ps_b[:, :], dg[:, :],
            AP(img.tensor, (u + 14) * 38 + v, [[1444, 128], [38, 14], [1, 28]]),
            start=(t == 0), stop=(t == 120),
        )
    corr = corrpool.tile([128, 924], BF16)
    nc.vector.tensor_copy(out=corr[:, 0:392], in_=ps_a[:, :])
    nc.vector.tensor_copy(out=corr[:, 392:784], in_=ps_b[:, :])
    nc.gpsimd.memset(corr[:, 784:924], 0.0)
    corr_d = dscrpool.tile([136, 924], BF16, tag="corr_d")
    nc.sync.dma_start(
        out=AP(corr_d.tensor, 0, [[924, 128], [1, 924]]),
        in_=corr[:, :],
    )
    # zero tail rows (s25g shifted reads overrun into them)
    nc.sync.dma_start(
        out=AP(corr_d.tensor, 128 * 924, [[924, 8], [1, 924]]),
        in_=zpad[:, :],
    )
    return corr_d


def _conv_stage(nc, b, corr_d, out_d, P, C):
    logitsb = P["lgbpool"].tile([16, 128], F32)
    # sub-level software pipeline, depth 2: A(s+2) is issued before B(s)
    # so PE always has conv1 work while dup DMAs for the next B transfer.
    dups = {}
    for s in range(6):
        if s < 4:
            dups[s] = _conv_a(nc, b, s, corr_d, P, C)
        if s >= 2:
            _conv_b(nc, b, s - 2, dups.pop(s - 2), logitsb, P, C)
    _softmax_out(nc, b, logitsb, out_d, P, C)


def _conv_a(nc, b, sub, corr_d, P, C):
    """s25g im2col -> conv1 -> maxpool (raw) -> bias+relu -> shifted dups."""
    w1x4_sb = C["w1x4_sb"]; b1x4_sb = C["b1x4_sb"]
    # s25g[20*dy+4*dx+g, c*924+j] = corr[32*sub+8*g+c, dy*28+dx + j]
    s25g = P["s25pool"].tile([100, 7392], BF16)
    for dy in range(5):
        nc.gpsimd.dma_start(
            out=s25g[20 * dy : 20 * dy + 20, :],
            in_=AP(
                corr_d.tensor,
                sub * 32 * 924 + dy * 28,
                [[1, 5], [7392, 4], [1, 7392]],
            ),
        )
    px_all = P["pxpool"].tile([128, 2304], BF16)
    for c in range(8):
        for h in range(2):
            ps1 = P["pc1"].tile([128, 288], F32, tag="ps1")
            nc.tensor.matmul(
                ps1[:, :],
                w1x4_sb[0:100, :],
                AP(
                    s25g.tensor,
                    c * 924 + h * 336,
                    [[7392, 100], [28, 12], [1, 24]],
                ),
                start=True, stop=True,
            )
            # maxpool x-pairs straight off PSUM (bias/relu commute with max)
            nc.vector.tensor_reduce(
                out=px_all[:, c * 288 + h * 144 : c * 288 + h * 144 + 144],
                in_=AP(ps1.tensor, 0, [[288, 128], [24, 12], [2, 12], [1, 2]]),
                axis=AXIS.X,
                op=ALU.max,
            )
    # maxpool y-pairs then one bias+relu -> pooled [128=(4co+g), (c,12,12)]
    py_raw = P["pyrpool"].tile([128, 1152], BF16)
    nc.vector.tensor_max(
        py_raw[:, :],
        AP(px_all.tensor, 0, [[2304, 128], [24, 96], [1, 12]]),
        AP(px_all.tensor, 12, [[2304, 128], [24, 96], [1, 12]]),
    )
    py_all = P["pypool"].tile([128, 1204], BF16)
    nc.scalar.activation(
        py_all[:, 0:1152], py_raw[:, :], ACTF.Relu, bias=b1x4_sb[:, 0:1]
    )
    nc.gpsimd.memset(py_all[:, 1152:1204], 0.0)
    # shifted dups (partition remap to [32ci, ...]):
    # dup_A row-group r bakes dx=r (shift r elems); dup2 bakes dx=4
    dup_A = P["dupApool"].tile([128, 4612], BF16)
    dup2 = P["dup2pool"].tile([32, 4608], BF16)
    for r in range(4):
        nc.sync.dma_start(
            out=AP(
                dup_A.tensor,
                32 * r * 4612 + (4 - r),
                [[4612, 32], [1152, 4], [1, 1152]],
            ),
            in_=AP(py_all.tensor, 0, [[1204, 128], [1, 1152]]),
        )
    nc.sync.dma_start(
        out=AP(dup2.tensor, 0, [[4608, 32], [1152, 4], [1, 1152]]),
        in_=AP(py_all.tensor, 4, [[1204, 128], [1, 1152]]),
    )
    return dup_A, dup2


def _conv_b(nc, b, sub, dup_pair, logitsb, P, C):
    """conv2 (dy via rhs offset, dx via dup row-groups) -> pool -> conv3 -> GAP."""
    dup_A, dup2 = dup_pair
    w2dx_sb = C["w2dx_sb"]; w2x4_sb = C["w2x4_sb"]; b2p_sb = C["b2p_sb"]
    w3n_sb = C["w3n_sb"]; b3q_sb = C["b3q_sb"]

    l3 = P["l3pool"].tile([64, 512], BF16)
    for cc in range(4):
        ps2 = P["pc2"].tile([64, 512], F32, tag="ps2")
        for dy in range(5):
            nc.tensor.matmul(
                ps2[:, :],
                w2dx_sb[:, 64 * dy : 64 * dy + 64],
                AP(
                    dup_A.tensor,
                    4 + cc * 1152 + dy * 12,
                    [[4612, 128], [144, 8], [12, 8], [1, 8]],
                ),
                start=(dy == 0), stop=False,
            )
        for dy in range(5):
            nc.tensor.matmul(
                ps2[:, :],
                w2x4_sb[:, 64 * dy : 64 * dy + 64],
                AP(
                    dup2.tensor,
                    cc * 1152 + dy * 12,
                    [[4608, 32], [144, 8], [12, 8], [1, 8]],
                ),
                start=False, stop=(dy == 4),
                tile_position=(0, 0),
            )
        # bias+relu off PSUM on ACT, then maxpool 2x2 on DVE into l3
        o2 = P["o2pool"].tile([64, 512], BF16)
        nc.scalar.activation(o2[:, :], ps2[:, :], ACTF.Relu, bias=b2p_sb[:, 0:1])
        qx = P["qxpool"].tile([64, 256], BF16)
        nc.vector.tensor_reduce(
            out=qx[:, :],
            in_=AP(o2.tensor, 0, [[512, 64], [8, 64], [2, 4], [1, 2]]),
            axis=AXIS.X,
            op=ALU.max,
        )
        nc.vector.tensor_max(
            l3[:, cc * 128 : cc * 128 + 128],
            AP(qx.tensor, 0, [[256, 64], [32, 8], [8, 4], [1, 4]]),
            AP(qx.tensor, 4, [[256, 64], [32, 8], [8, 4], [1, 4]]),
        )

    # conv3: 9 taps, K=64, one PSUM region
    ps3 = P["pc3"].tile([16, 128], F32, tag="ps3")
    for t in range(9):
        dy, dx = t // 3, t % 3
        nc.tensor.matmul(
            ps3[0:10, :],
            w3n_sb[0:64, 10 * t : 10 * t + 10],
            AP(l3.tensor, dy * 4 + dx, [[512, 64], [16, 32], [4, 2], [1, 2]]),
            start=(t == 0), stop=(t == 8),
        )
    # relu(0.25*x + 0.25*b3) then sum 4 pix = GAP of relu(x+b3)
    ga = P["smpool"].tile([16, 128], F32, tag="ga")
    nc.scalar.activation(
        ga[0:10, :], ps3[0:10, :], ACTF.Relu, bias=b3q_sb[0:10, 0:1], scale=0.25
    )
    nc.vector.tensor_reduce(
        out=logitsb[0:10, sub * 32 : sub * 32 + 32],
        in_=AP(ga.tensor, 0, [[128, 10], [4, 32], [1, 4]]),
        axis=AXIS.X,
        op=ALU.add,
    )


def _softmax_out(nc, b, logitsb, out_d, P, C):
    smpool = P["smpool"]
    psT = P["pc3"].tile([128, 16], F32, tag="psT")
    nc.tensor.transpose(psT[:, 0:10], logitsb[0:10, :], C["ident10"][0:10, 0:10])
    mx = smpool.tile([128, 1], F32, tag="mx")
    nc.vector.reduce_max(mx[:, :], psT[:, 0:10], axis=AXIS.X)
    hs = smpool.tile([128, 16], F32, tag="hs")
    nc.vector.tensor_scalar(hs[:, 0:10], psT[:, 0:10], mx[:, 0:1], None, ALU.subtract)
    ex = smpool.tile([128, 16], F32, tag="ex")
    nc.scalar.activation(ex[:, 0:10], hs[:, 0:10], ACTF.Exp)
    sm = smpool.tile([128, 1], F32, tag="sm")
    nc.vector.reduce_sum(sm[:, :], ex[:, 0:10], axis=AXIS.X)
    lsm = smpool.tile([128, 1], F32, tag="lsm")
    nc.scalar.activation(lsm[:, :], sm[:, :], ACTF.Ln)
    outt = smpool.tile([128, 16], F32, tag="outt")
    nc.vector.tensor_scalar(outt[:, 0:10], hs[:, 0:10], lsm[:, 0:1], None, ALU.subtract)
    nc.sync.dma_start(
        out=AP(out_d, b * 1280, [[10, 128], [1, 10]]),
        in_=outt[:, 0:10],
    )


_CACHE = {}


def _get_nc(b_core):
    if b_core not in _CACHE:
        nc = bacc.Bacc(
            "TRN2",
            target_bir_lowering=False,
            debug=False,
            num_devices=N_CORES,
            num_swdge_queues=2,
        )
        _build(nc, b_core)
        nc.compile()
        _CACHE[b_core] = nc
    return _CACHE[b_core]


def _prep_inputs(inputs):
    import ml_dtypes

    bf16 = ml_dtypes.bfloat16
    w1 = np.asarray(inputs["w1"], dtype=np.float32).reshape(32, 25)
    w2 = np.asarray(inputs["w2"], dtype=np.float32).reshape(64, 32, 5, 5)
    w3 = np.asarray(inputs["w3"], dtype=np.float32).reshape(10, 64, 9)
    b1 = np.asarray(inputs["b1"], dtype=np.float32)
    b2 = np.asarray(inputs["b2"], dtype=np.float32)
    b3 = np.asarray(inputs["b3"], dtype=np.float32)

    # conv1: w1x4[4*t+g, 4*co+g] = w1[co, t]
    w1x4 = np.zeros((100, 128), dtype=np.float32)
    for t in range(25):
        for g in range(4):
            w1x4[4 * t + g, 4 * np.arange(32) + g] = w1[:, t]
    b1x4 = np.zeros((128, 1), dtype=np.float32)
    for co in range(32):
        for g in range(4):
            b1x4[4 * co + g, 0] = b1[co]

    # conv2: w2dx[32*r+ci, 64*dy+co] = w2[co, ci, dy, r] (r=dx 0..3)
    w2dx = np.zeros((128, 320), dtype=np.float32)
    for r in range(4):
        for dy in range(5):
            w2dx[32 * r : 32 * r + 32, 64 * dy : 64 * dy + 64] = w2[:, :, dy, r].T
    # w2x4[ci, 64*dy+co] = w2[co, ci, dy, 4]
    w2x4 = np.zeros((32, 320), dtype=np.float32)
    for dy in range(5):
        w2x4[:, 64 * dy : 64 * dy + 64] = w2[:, :, dy, 4].T

    # conv3: w3n[ci, 10*t+co] = w3[co, ci, t]
    w3n = np.zeros((64, 96), dtype=np.float32)
    for t in range(9):
        w3n[:, 10 * t : 10 * t + 10] = w3[:, :, t].T
    b3q = np.zeros((16, 1), dtype=np.float32)
    b3q[0:10, 0] = 0.25 * b3

    return dict(
        identp=np.eye(128, dtype=bf16),
        ident10p=np.eye(16, dtype=np.float32),
        w1x4=w1x4.astype(bf16),
        b1x4=b1x4,
        w2dx=w2dx.astype(bf16),
        w2x4=w2x4.astype(bf16),
        b2p=b2.reshape(64, 1),
        w3n=w3n.astype(bf16),
        b3q=b3q,
    )


def _run(inputs, b_core=B_CORE, trace=False):
    x = np.ascontiguousarray(np.asarray(inputs["x"], dtype=np.float32))
    consts = _prep_inputs(inputs)
    nc = _get_nc(b_core)
    in_maps = [
        {"x": x[i * b_core : (i + 1) * b_core], **consts} for i in range(N_CORES)
    ]
    res = run_bass_kernel_spmd(nc, in_maps, core_ids=list(range(N_CORES)), trace=trace)
    out = np.concatenate([res.results[i]["out"] for i in range(N_CORES)], axis=0)
    return out.astype(np.float32), res


def kernel(**inputs) -> np.ndarray:
    out, _ = _run(inputs)
    return out
